# revision 4
# baseline (speedup 1.0000x reference)
"""Distributed MultiHeadAttention kernel for 8 TRN2 NeuronCores.

Problem: B=4, S=2048, D=1024, H=16, DH=64, fp32, full (non-causal) attention.

Sharding: each core owns (batch b = core//2, query-half qh = core%2):
1024 queries x full 2048-key sequence, all 16 heads. K/V projections are
computed per-core for the full sequence of its batch (2x duplicated across
the query-half pair of cores); Q and output projections cover only the
core's 1024 queries. No cross-core communication; host concatenates the
8 [1024, 1024] output slices.

Layout strategy (per core):
- All matmuls run in float32r (full PE rate at N>=256, ~1.5e-4 rel err).
- x is kept transposed xT [D, S] resident in SBUF; projections produce
  qT/kT [dout_pair, tok] (transposed, head-pair on partitions) and
  V [keys, dout] (natural) directly from xT.
- Scores are computed transposed sT[keys, q] with a row-tiled head pair:
  head h0 contracts on partitions 0-63, h1 on 64-127 concurrently.
- exp() runs on the scalar engine PSUM->SBUF with no max-subtraction
  (scores ~ N(0,1), max << 80).
- PV uses the "garbage rows" trick: lhsT = [V_h0 | V_h1] [128k, 128],
  rhs = PT_h per head; half of each PSUM result is used, so both heads
  land on their correct partitions without col tiling.
- Softmax sums: DVE accumulates S_acc += PT per key chunk; a single
  ones-column matmul reduces partitions -> [1, 512]; reciprocal; a K=1
  ones-row matmul broadcasts it to [128, 512]; DVE multiplies into the
  attention output (aoT, f32r), which feeds the output projection.
"""

import numpy as np
import concourse.bass as bass
import concourse.mybir as mybir
from concourse.tile import TileContext
from concourse.bass_utils import run_bass_kernel_spmd

F32 = mybir.dt.float32
F32R = mybir.dt.float32r

B, S, D, H = 4, 2048, 1024, 16
DH = D // H
N_CORES = 8
NQ = S * B // N_CORES      # 1024 queries per core
PAIRS = 8                  # head pairs (128 dout each)
DINC = 8                   # 128-wide din chunks
KC = S // 128              # 16 key chunks
QT = NQ // 512             # 2 query tiles
NBLK = 4                   # V-projection blocks (2 pairs each)

_ws_counter = 0


def _split_multi_waits(nc):
    """walrus in this env rejects >1 sem wait per instruction; hoist extras
    onto same-engine standalone semaphore-wait instructions."""
    global _ws_counter
    f = nc.m.functions[0]
    for bb in f.blocks:
        insts = bb.instructions  # live list
        i = 0
        while i < len(insts):
            inst = insts[i]
            si = inst.sync_info
            waits = list(si.on_wait) if si is not None and si.on_wait else []
            if len(waits) > 1:
                eng = getattr(inst, "engine", None)
                assert eng is not None and eng in nc.engines, (
                    f"multi-wait on non-engine inst {inst.name} ({type(inst).__name__})"
                )
                for w in waits[:-1]:
                    _ws_counter += 1
                    ev = mybir.InstEventSemaphore(
                        name=f"I-wsplit-{_ws_counter}", ins=[], outs=[]
                    )
                    ev.engine = eng
                    ev.sync_info = mybir.SyncInfo(on_wait=[w], on_update=[])
                    nc.register_instruction(ev, overwrite=True)
                    insts.insert(i, ev)
                    i += 1
                inst.sync_info = mybir.SyncInfo(
                    on_wait=[waits[-1]], on_update=list(si.on_update or [])
                )
            i += 1


def _r(ap):
    return ap.bitcast(F32R)


def build_bass(qh: int):
    """One SPMD program; qh (query half) differs between even/odd cores, so
    we build two variants and run them as one 8-core launch... (actually we
    encode qh by slicing xT columns; the program differs only in a constant
    column offset, so build per qh)."""
    nc = bass.Bass()
    XT = nc.declare_dram_parameter("XT", [D, S], F32, isOutput=False)
    WQP = nc.declare_dram_parameter("WQP", [PAIRS, 128, 1024], F32, isOutput=False)
    WKP = nc.declare_dram_parameter("WKP", [PAIRS, 128, 1024], F32, isOutput=False)
    WVP = nc.declare_dram_parameter("WVP", [NBLK, 128, 2048], F32, isOutput=False)
    WOP = nc.declare_dram_parameter("WOP", [2, 128, 4096], F32, isOutput=False)
    BQK = nc.declare_dram_parameter("BQK", [128, 16], F32, isOutput=False)
    BVB = nc.declare_dram_parameter("BVB", [128, 1024], F32, isOutput=False)
    ONESC = nc.declare_dram_parameter("ONESC", [128, 1], F32, isOutput=False)
    ONESR = nc.declare_dram_parameter("ONESR", [1, 128], F32, isOutput=False)
    Y = nc.declare_dram_parameter("Y", [NQ, D], F32, isOutput=True)

    qcol0 = qh * NQ  # column offset of our queries inside xT

    with TileContext(nc) as tc:
        with (
            tc.tile_pool(name="sb", bufs=1) as sb,
            tc.tile_pool(name="ps", bufs=1, space="PSUM") as ps,
        ):
            # ---- constants / resident tensors
            ones_c = sb.tile([128, 1], F32R, tag="ones_c")
            ones_r = sb.tile([1, 128], F32R, tag="ones_r")
            bqk = sb.tile([128, 16], F32, tag="bqk")
            bvb = sb.tile([128, 1024], F32, tag="bvb")
            nc.sync.dma_start(out=ones_c[:, :], in_=ONESC[:, :].bitcast(F32R))
            nc.sync.dma_start(out=ones_r[:, :], in_=ONESR[:, :].bitcast(F32R))
            nc.sync.dma_start(out=bqk[:, :], in_=BQK[:, :])
            nc.sync.dma_start(out=bvb[:, :], in_=BVB[:, :])

            xt = []
            for d in range(DINC):
                t = sb.tile([128, S], F32R, tag=f"xt{d}")
                nc.sync.dma_start(out=t[:, :], in_=XT[d * 128:(d + 1) * 128, :].bitcast(F32R))
                xt.append(t)

            aot = [sb.tile([128, NQ], F32R, tag=f"ao{j}", name=f"ao{j}") for j in range(PAIRS)]

            # ---- main loop over 2-pair blocks
            for blk in range(NBLK):
                # V-projection for this block: V_blk [2048 keys, 256]
                wv_t = sb.tile([128, 2048], F32R, tag="wstream", bufs=1, name="wv_t")
                nc.sync.dma_start(out=wv_t[:, :], in_=WVP[blk, :, :].bitcast(F32R))
                v4 = [sb.tile([128, 1024], F32R, tag=f"v{t}", name=f"v{t}_{blk}") for t in range(4)]
                for kc in range(KC):
                    vps = ps.tile([128, 256], F32, tag="ps_proj", bufs=2)
                    for d in range(DINC):
                        nc.tensor.matmul(
                            vps[:, :],
                            xt[d][:, kc * 128:(kc + 1) * 128],
                            wv_t[:, d * 256:(d + 1) * 256],
                            start=(d == 0), stop=(d == DINC - 1),
                        )
                    with nc.allow_low_precision(reason="f32r rounding"):
                        nc.vector.tensor_add(
                            v4[kc // 4][:, (kc % 4) * 256:((kc % 4) + 1) * 256],
                            vps[:, :],
                            bvb[:, blk * 256:(blk + 1) * 256],
                        )

                for jj in range(2):
                    j = blk * 2 + jj
                    wq_t = sb.tile([128, 1024], F32R, tag="wq", bufs=2)
                    wk_t = sb.tile([128, 1024], F32R, tag="wk", bufs=2)
                    nc.sync.dma_start(out=wq_t[:, :], in_=WQP[j, :, :].bitcast(F32R))
                    nc.sync.dma_start(out=wk_t[:, :], in_=WKP[j, :, :].bitcast(F32R))

                    # Q-projection: qT pair [128, 1024] for our queries
                    qt_t = sb.tile([128, NQ], F32R, tag="qt", bufs=2)
                    for q2 in range(QT):
                        qps = ps.tile([128, 512], F32, tag="ps_proj", bufs=2)
                        for d in range(DINC):
                            nc.tensor.matmul(
                                qps[:, :],
                                wq_t[:, d * 128:(d + 1) * 128],
                                xt[d][:, qcol0 + q2 * 512: qcol0 + (q2 + 1) * 512],
                                start=(d == 0), stop=(d == DINC - 1),
                            )
                        nc.scalar.activation(
                            qt_t[:, q2 * 512:(q2 + 1) * 512], qps[:, :],
                            mybir.ActivationFunctionType.Identity,
                            bias=bqk[:, 2 * j:2 * j + 1], scale=1.0,
                        )

                    # K-projection: kT pair [128, 2048] full sequence
                    kt_t = sb.tile([128, S], F32R, tag="kt", bufs=1)
                    for tt in range(4):
                        kps = ps.tile([128, 512], F32, tag="ps_proj", bufs=2)
                        for d in range(DINC):
                            nc.tensor.matmul(
                                kps[:, :],
                                wk_t[:, d * 128:(d + 1) * 128],
                                xt[d][:, tt * 512:(tt + 1) * 512],
                                start=(d == 0), stop=(d == DINC - 1),
                            )
                        nc.scalar.activation(
                            kt_t[:, tt * 512:(tt + 1) * 512], kps[:, :],
                            mybir.ActivationFunctionType.Identity,
                            bias=bqk[:, 2 * j + 1:2 * j + 2], scale=1.0,
                        )

                    # attention for this pair
                    for q2 in range(QT):
                        psA = ps.tile([128, 512], F32, tag="ps_pv", bufs=2)
                        psB = ps.tile([128, 512], F32, tag="ps_pv", bufs=2)
                        sacc = sb.tile([128, 1024], F32R, tag="sacc", bufs=2)
                        qsl = slice(q2 * 512, (q2 + 1) * 512)
                        for kc in range(KC):
                            pss0 = ps.tile([128, 512], F32, tag="ps_s", bufs=4)
                            pss1 = ps.tile([128, 512], F32, tag="ps_s", bufs=4)
                            ksl = slice(kc * 128, (kc + 1) * 128)
                            nc.tensor.matmul(
                                pss0[:, :], kt_t[0:64, ksl], qt_t[0:64, qsl],
                                start=True, stop=True,
                            )
                            nc.tensor.matmul(
                                pss1[:, :], kt_t[64:128, ksl], qt_t[64:128, qsl],
                                start=True, stop=True,
                            )
                            pt = sb.tile([128, 1024], F32R, tag="pt", bufs=2)
                            nc.scalar.activation(
                                pt[:, 0:512], pss0[:, :],
                                mybir.ActivationFunctionType.Exp,
                            )
                            nc.scalar.activation(
                                pt[:, 512:1024], pss1[:, :],
                                mybir.ActivationFunctionType.Exp,
                            )
                            vpair = v4[kc // 4][:, (kc % 4) * 256 + jj * 128:
                                               (kc % 4) * 256 + (jj + 1) * 128]
                            nc.tensor.matmul(
                                psA[:, :], vpair, pt[:, 0:512],
                                start=(kc == 0), stop=(kc == KC - 1),
                            )
                            nc.tensor.matmul(
                                psB[:, :], vpair, pt[:, 512:1024],
                                start=(kc == 0), stop=(kc == KC - 1),
                            )
                            with nc.allow_low_precision(reason="f32r rounding"):
                                if kc == 0:
                                    nc.vector.tensor_copy(sacc[:, :], pt[:, :])
                                else:
                                    nc.vector.tensor_add(sacc[:, :], sacc[:, :], pt[:, :])

                        # softmax tail: partition-reduce, reciprocal, broadcast, scale
                        psm0 = ps.tile([1, 512], F32, tag="ps_s", bufs=4)
                        psm1 = ps.tile([1, 512], F32, tag="ps_s", bufs=4)
                        nc.tensor.matmul(psm0[:, :], ones_c[:, :], sacc[:, 0:512],
                                         start=True, stop=True)
                        nc.tensor.matmul(psm1[:, :], ones_c[:, :], sacc[:, 512:1024],
                                         start=True, stop=True)
                        recip = sb.tile([1, 1024], F32R, tag="recip", bufs=2)
                        with nc.allow_low_precision(reason="f32r rounding"):
                            nc.vector.reciprocal(recip[:, 0:512], psm0[:, :])
                            nc.vector.reciprocal(recip[:, 512:1024], psm1[:, :])
                        psbc0 = ps.tile([128, 512], F32, tag="ps_s", bufs=4)
                        psbc1 = ps.tile([128, 512], F32, tag="ps_s", bufs=4)
                        nc.tensor.matmul(psbc0[:, :], ones_r[:, :], recip[:, 0:512],
                                         start=True, stop=True)
                        nc.tensor.matmul(psbc1[:, :], ones_r[:, :], recip[:, 512:1024],
                                         start=True, stop=True)
                        bc = sb.tile([128, 1024], F32, tag="bc", bufs=2)
                        nc.scalar.copy(out=bc[:, 0:512], in_=psbc0[:, :])
                        nc.scalar.copy(out=bc[:, 512:1024], in_=psbc1[:, :])
                        with nc.allow_low_precision(reason="f32r rounding"):
                            nc.vector.tensor_mul(
                                aot[j][0:64, qsl], psA[0:64, :], bc[0:64, 0:512]
                            )
                            nc.vector.tensor_mul(
                                aot[j][64:128, qsl], psB[64:128, :], bc[64:128, 512:1024]
                            )

            # ---- output projection: Y[tok, dout] = aoT.T @ woT
            for nt in range(2):
                wo_t = sb.tile([128, 4096], F32R, tag="wstream", bufs=1, name="wo_t")
                nc.sync.dma_start(out=wo_t[:, :], in_=WOP[nt, :, :].bitcast(F32R))
                for tc_ in range(8):
                    yps = ps.tile([128, 512], F32, tag="ps_proj", bufs=2)
                    for j in range(PAIRS):
                        nc.tensor.matmul(
                            yps[:, :],
                            aot[j][:, tc_ * 128:(tc_ + 1) * 128],
                            wo_t[:, j * 512:(j + 1) * 512],
                            start=(j == 0), stop=(j == PAIRS - 1),
                        )
                    y_sb = sb.tile([128, 512], F32, tag="y", bufs=2)
                    nc.scalar.copy(out=y_sb[:, :], in_=yps[:, :])
                    nc.sync.dma_start(
                        out=Y[tc_ * 128:(tc_ + 1) * 128, nt * 512:(nt + 1) * 512],
                        in_=y_sb[:, :],
                    )

    _split_multi_waits(nc)
    return nc


_nc_cache = {}
_last_results = None


def _get_nc(qh):
    if qh not in _nc_cache:
        _nc_cache[qh] = build_bass(qh)
    return _nc_cache[qh]


def _prep_weights(wq, bq, wk, bk, wv, bv, wo):
    wqT = np.ascontiguousarray(wq.T) * np.float32(1.0 / np.sqrt(DH))
    wkT = np.ascontiguousarray(wk.T)
    wvT = np.ascontiguousarray(wv.T)
    woT = np.ascontiguousarray(wo.T)
    # WQP[j, p, (d m)] = wqT[d*128+p, j*128+m]
    A = wqT.reshape(DINC, 128, PAIRS, 128)
    WQP = np.ascontiguousarray(A.transpose(2, 1, 0, 3).reshape(PAIRS, 128, 1024))
    A = wkT.reshape(DINC, 128, PAIRS, 128)
    WKP = np.ascontiguousarray(A.transpose(2, 1, 0, 3).reshape(PAIRS, 128, 1024))
    # WVP[blk, p, (d n)] = wvT[d*128+p, blk*256+n]
    A = wvT.reshape(DINC, 128, NBLK, 256)
    WVP = np.ascontiguousarray(A.transpose(2, 1, 0, 3).reshape(NBLK, 128, 2048))
    # WOP[nt, p, (j n)] = woT[j*128+p, nt*512+n]
    A = woT.reshape(PAIRS, 128, 2, 512)
    WOP = np.ascontiguousarray(A.transpose(2, 1, 0, 3).reshape(2, 128, 4096))
    bqs = (bq * np.float32(1.0 / np.sqrt(DH))).reshape(PAIRS, 128)
    bkr = bk.reshape(PAIRS, 128)
    BQK = np.empty((128, 16), np.float32)
    for jx in range(PAIRS):
        BQK[:, 2 * jx] = bqs[jx]
        BQK[:, 2 * jx + 1] = bkr[jx]
    BVB = np.ascontiguousarray(np.tile(bv.reshape(1, D), (128, 1)))
    return WQP, WKP, WVP, WOP, BQK, BVB


def kernel(x_input, wq, bq, wk, bk, wv, bv, wo, bo):
    x_input = np.asarray(x_input, dtype=np.float32)
    wq, bq = np.asarray(wq, np.float32), np.asarray(bq, np.float32)
    wk, bk = np.asarray(wk, np.float32), np.asarray(bk, np.float32)
    wv, bv = np.asarray(wv, np.float32), np.asarray(bv, np.float32)
    wo, bo = np.asarray(wo, np.float32), np.asarray(bo, np.float32)

    WQP, WKP, WVP, WOP, BQK, BVB = _prep_weights(wq, bq, wk, bk, wv, bv, wo)
    ONESC = np.ones((128, 1), np.float32)
    ONESR = np.ones((1, 128), np.float32)

    shared = {
        "WQP": WQP, "WKP": WKP, "WVP": WVP, "WOP": WOP,
        "BQK": BQK, "BVB": BVB, "ONESC": ONESC, "ONESR": ONESR,
    }
    xTs = [np.ascontiguousarray(x_input[b].T) for b in range(B)]

    # qh is baked into the program; all 8 cores must run ONE program under
    # SPMD, so instead bake qh=0 and shift each odd core's xT columns so its
    # queries sit at columns 0..1023 -- NO: that would break K/V (full seq).
    # Instead: build with qh as a parameter and run even/odd cores in one
    # launch is impossible under one NEFF; so we pass per-core xT where the
    # query half is ALWAYS columns [0,1024) by ROTATING the sequence for odd
    # cores, and un-rotate the keys... also breaks nothing: attention is
    # permutation-equivariant in keys! Rotating the key/token axis by 1024
    # for odd cores leaves softmax(QK^T)V unchanged per query; queries then
    # occupy columns 0..1023 of the rotated xT. Output rows are our queries
    # in rotated order = original columns 1024..2047. So: one program
    # (qh=0), odd cores get np.roll(xT, -1024, axis=1).
    nc = _get_nc(0)
    in_maps = []
    for c in range(N_CORES):
        b, qh = c // 2, c % 2
        xt = xTs[b] if qh == 0 else np.ascontiguousarray(
            np.roll(xTs[b], -NQ, axis=1))
        m = dict(shared)
        m["XT"] = xt
        in_maps.append(m)

    res = run_bass_kernel_spmd(nc, in_maps, list(range(N_CORES)))
    global _last_results
    _last_results = res

    out = np.empty((B, S, D), np.float32)
    for c in range(N_CORES):
        b, qh = c // 2, c % 2
        out[b, qh * NQ:(qh + 1) * NQ, :] = res.results[c]["Y"]
    out += bo.reshape(1, 1, D)
    return out


# revision 7
# speedup vs baseline: 1.8306x; 1.8306x over previous
"""Distributed MultiHeadAttention kernel for 8 TRN2 NeuronCores.

Problem: B=4, S=2048, D=1024, H=16, DH=64, fp32, full (non-causal) attention.

Sharding: each core owns (batch b = core//2, query-half qh = core%2):
1024 queries x full 2048-key sequence, all 16 heads. K/V projections are
computed per-core for the full sequence of its batch (2x duplicated across
the query-half pair of cores); Q and output projections cover only the
core's 1024 queries. No cross-core communication; host concatenates the
8 [1024, 1024] output slices.

Layout strategy (per core):
- All matmuls run in float32r (full PE rate at N>=256, ~1.5e-4 rel err).
- x is kept transposed xT [D, S] resident in SBUF; projections produce
  qT/kT [dout_pair, tok] (transposed, head-pair on partitions) and
  V [keys, dout] (natural) directly from xT.
- Scores are computed transposed sT[keys, q] with a row-tiled head pair:
  head h0 contracts on partitions 0-63, h1 on 64-127 concurrently.
- exp() runs on the scalar engine PSUM->SBUF with no max-subtraction
  (scores ~ N(0,1), max << 80).
- PV uses the "garbage rows" trick: lhsT = [V_h0 | V_h1] [128k, 128],
  rhs = PT_h per head; half of each PSUM result is used, so both heads
  land on their correct partitions without col tiling.
- Softmax sums: DVE accumulates S_acc += PT per key chunk; a single
  ones-column matmul reduces partitions -> [1, 512]; reciprocal; a K=1
  ones-row matmul broadcasts it to [128, 512]; DVE multiplies into the
  attention output (aoT, f32r), which feeds the output projection.
"""

import numpy as np
import concourse.bass as bass
import concourse.mybir as mybir
from concourse.tile import TileContext
from concourse.bass_utils import run_bass_kernel_spmd

F32 = mybir.dt.float32
F32R = mybir.dt.float32r

B, S, D, H = 4, 2048, 1024, 16
DH = D // H
N_CORES = 8
NQ = S * B // N_CORES      # 1024 queries per core
PAIRS = 8                  # head pairs (128 dout each)
DINC = 8                   # 128-wide din chunks
KC = S // 128              # 16 key chunks
QT = NQ // 512             # 2 query tiles
NBLK = 4                   # V-projection blocks (2 pairs each)

_ws_counter = 0


def _split_multi_waits(nc):
    """walrus in this env rejects >1 sem wait per instruction; hoist extras
    onto same-engine standalone semaphore-wait instructions."""
    global _ws_counter
    f = nc.m.functions[0]
    for bb in f.blocks:
        insts = bb.instructions  # live list
        i = 0
        while i < len(insts):
            inst = insts[i]
            si = inst.sync_info
            waits = list(si.on_wait) if si is not None and si.on_wait else []
            if len(waits) > 1:
                eng = getattr(inst, "engine", None)
                assert eng is not None and eng in nc.engines, (
                    f"multi-wait on non-engine inst {inst.name} ({type(inst).__name__})"
                )
                for w in waits[:-1]:
                    _ws_counter += 1
                    ev = mybir.InstEventSemaphore(
                        name=f"I-wsplit-{_ws_counter}", ins=[], outs=[]
                    )
                    ev.engine = eng
                    ev.sync_info = mybir.SyncInfo(on_wait=[w], on_update=[])
                    nc.register_instruction(ev, overwrite=True)
                    insts.insert(i, ev)
                    i += 1
                inst.sync_info = mybir.SyncInfo(
                    on_wait=[waits[-1]], on_update=list(si.on_update or [])
                )
            i += 1


def _r(ap):
    return ap.bitcast(F32R)


def build_bass(qh: int):
    """One SPMD program; qh (query half) differs between even/odd cores, so
    we build two variants and run them as one 8-core launch... (actually we
    encode qh by slicing xT columns; the program differs only in a constant
    column offset, so build per qh)."""
    nc = bass.Bass()
    XT = nc.declare_dram_parameter("XT", [D, S], F32, isOutput=False)
    WQP = nc.declare_dram_parameter("WQP", [PAIRS, 128, 1024], F32, isOutput=False)
    WKP = nc.declare_dram_parameter("WKP", [PAIRS, 128, 1024], F32, isOutput=False)
    WVP = nc.declare_dram_parameter("WVP", [NBLK, 128, 2048], F32, isOutput=False)
    WOP = nc.declare_dram_parameter("WOP", [2, 128, 4096], F32, isOutput=False)
    BQK = nc.declare_dram_parameter("BQK", [128, 16], F32, isOutput=False)
    BVB = nc.declare_dram_parameter("BVB", [128, 1024], F32, isOutput=False)
    ONES2D = nc.declare_dram_parameter("ONES2D", [128, 128], F32, isOutput=False)
    Y = nc.declare_dram_parameter("Y", [NQ, D], F32, isOutput=True)

    qcol0 = qh * NQ  # column offset of our queries inside xT

    with TileContext(nc) as tc:
        with (
            tc.tile_pool(name="sb", bufs=1) as sb,
            tc.tile_pool(name="ps", bufs=1, space="PSUM") as ps,
        ):
            # ---- constants / resident tensors
            ones2d = sb.tile([128, 128], F32R, tag="ones2d")
            bqk = sb.tile([128, 16], F32, tag="bqk")
            bvb = sb.tile([128, 1024], F32, tag="bvb")
            nc.sync.dma_start(out=ones2d[:, :], in_=ONES2D[:, :].bitcast(F32R))
            nc.sync.dma_start(out=bqk[:, :], in_=BQK[:, :])
            nc.sync.dma_start(out=bvb[:, :], in_=BVB[:, :])

            xt = []
            for d in range(DINC):
                t = sb.tile([128, S], F32R, tag=f"xt{d}")
                nc.sync.dma_start(out=t[:, :], in_=XT[d * 128:(d + 1) * 128, :].bitcast(F32R))
                xt.append(t)

            aot = [sb.tile([128, NQ], F32R, tag=f"ao{j}", name=f"ao{j}") for j in range(PAIRS)]

            # ---- main loop over 2-pair blocks
            for blk in range(NBLK):
                # V-projection for this block: V_blk [2048 keys, 256]
                wv_t = sb.tile([128, 2048], F32R, tag="wstream", bufs=1, name="wv_t")
                nc.sync.dma_start(out=wv_t[:, :], in_=WVP[blk, :, :].bitcast(F32R))
                v4 = [sb.tile([128, 1024], F32R, tag=f"v{t}", name=f"v{t}_{blk}") for t in range(4)]
                for kc in range(KC):
                    vps = ps.tile([128, 256], F32, tag="ps_proj", bufs=2)
                    for d in range(DINC):
                        nc.tensor.matmul(
                            vps[:, :],
                            xt[d][:, kc * 128:(kc + 1) * 128],
                            wv_t[:, d * 256:(d + 1) * 256],
                            start=(d == 0), stop=(d == DINC - 1),
                        )
                    with nc.allow_low_precision(reason="f32r rounding"):
                        nc.vector.tensor_add(
                            v4[kc // 4][:, (kc % 4) * 256:((kc % 4) + 1) * 256],
                            vps[:, :],
                            bvb[:, blk * 256:(blk + 1) * 256],
                        )

                for jj in range(2):
                    j = blk * 2 + jj
                    wq_t = sb.tile([128, 1024], F32R, tag="wq", bufs=2)
                    wk_t = sb.tile([128, 1024], F32R, tag="wk", bufs=2)
                    nc.sync.dma_start(out=wq_t[:, :], in_=WQP[j, :, :].bitcast(F32R))
                    nc.sync.dma_start(out=wk_t[:, :], in_=WKP[j, :, :].bitcast(F32R))

                    # Q-projection: qT pair [128, 1024] for our queries
                    qt_t = sb.tile([128, NQ], F32R, tag="qt", bufs=2)
                    for q2 in range(QT):
                        qps = ps.tile([128, 512], F32, tag="ps_proj", bufs=2)
                        for d in range(DINC):
                            nc.tensor.matmul(
                                qps[:, :],
                                wq_t[:, d * 128:(d + 1) * 128],
                                xt[d][:, qcol0 + q2 * 512: qcol0 + (q2 + 1) * 512],
                                start=(d == 0), stop=(d == DINC - 1),
                            )
                        with nc.allow_low_precision(reason="f32r rounding"):
                            nc.vector.tensor_scalar_add(
                                qt_t[:, q2 * 512:(q2 + 1) * 512], qps[:, :],
                                bqk[:, 2 * j:2 * j + 1],
                            )

                    # K-projection: kT pair [128, 2048] full sequence
                    kt_t = sb.tile([128, S], F32R, tag="kt", bufs=2)
                    for tt in range(4):
                        kps = ps.tile([128, 512], F32, tag="ps_proj", bufs=2)
                        for d in range(DINC):
                            nc.tensor.matmul(
                                kps[:, :],
                                wk_t[:, d * 128:(d + 1) * 128],
                                xt[d][:, tt * 512:(tt + 1) * 512],
                                start=(d == 0), stop=(d == DINC - 1),
                            )
                        with nc.allow_low_precision(reason="f32r rounding"):
                            nc.vector.tensor_scalar_add(
                                kt_t[:, tt * 512:(tt + 1) * 512], kps[:, :],
                                bqk[:, 2 * j + 1:2 * j + 2],
                            )

                    # attention for this pair
                    for q2 in range(QT):
                        psA = ps.tile([128, 512], F32, tag="ps_pv", bufs=2)
                        psB = ps.tile([128, 512], F32, tag="ps_pv", bufs=2)
                        sacc = sb.tile([128, 1024], F32R, tag="sacc", bufs=2)
                        qsl = slice(q2 * 512, (q2 + 1) * 512)
                        for kc in range(KC):
                            pss = ps.tile([128, 1024], F32, tag="ps_s", bufs=2)
                            ksl = slice(kc * 128, (kc + 1) * 128)
                            nc.tensor.matmul(
                                pss[:, 0:512], kt_t[0:64, ksl], qt_t[0:64, qsl],
                                start=True, stop=True,
                            )
                            nc.tensor.matmul(
                                pss[:, 512:1024], kt_t[64:128, ksl], qt_t[64:128, qsl],
                                start=True, stop=True,
                            )
                            pt = sb.tile([128, 1024], F32R, tag="pt", bufs=3)
                            nc.scalar.activation(
                                pt[:, :], pss[:, :],
                                mybir.ActivationFunctionType.Exp,
                            )
                            vpair = v4[kc // 4][:, (kc % 4) * 256 + jj * 128:
                                               (kc % 4) * 256 + (jj + 1) * 128]
                            nc.tensor.matmul(
                                psA[:, :], vpair, pt[:, 0:512],
                                start=(kc == 0), stop=(kc == KC - 1),
                            )
                            nc.tensor.matmul(
                                psB[:, :], vpair, pt[:, 512:1024],
                                start=(kc == 0), stop=(kc == KC - 1),
                            )
                            with nc.allow_low_precision(reason="f32r rounding"):
                                if kc == 0:
                                    nc.vector.tensor_copy(sacc[:, :], pt[:, :])
                                else:
                                    nc.vector.tensor_add(sacc[:, :], sacc[:, :], pt[:, :])

                        # softmax tail: fused reduce+broadcast matmul, 1/x via exp(-ln)
                        psbc = ps.tile([128, 1024], F32, tag="ps_s", bufs=2)
                        nc.tensor.matmul(psbc[:, 0:512], ones2d[:, :], sacc[:, 0:512],
                                         start=True, stop=True)
                        nc.tensor.matmul(psbc[:, 512:1024], ones2d[:, :], sacc[:, 512:1024],
                                         start=True, stop=True)
                        lnt = sb.tile([128, 1024], F32, tag="lnt", bufs=1)
                        nc.scalar.activation(lnt[:, :], psbc[:, :],
                                             mybir.ActivationFunctionType.Ln)
                        bcr = sb.tile([128, 1024], F32, tag="bcr", bufs=1)
                        nc.scalar.activation(bcr[:, :], lnt[:, :],
                                             mybir.ActivationFunctionType.Exp,
                                             scale=-1.0)
                        with nc.allow_low_precision(reason="f32r rounding"):
                            nc.vector.tensor_mul(
                                aot[j][0:64, qsl], psA[0:64, :], bcr[0:64, 0:512]
                            )
                            nc.vector.tensor_mul(
                                aot[j][64:128, qsl], psB[64:128, :], bcr[64:128, 512:1024]
                            )

            # ---- output projection: Y[tok, dout] = aoT.T @ woT
            for nt in range(2):
                wo_t = sb.tile([128, 4096], F32R, tag="wstream", bufs=1, name="wo_t")
                nc.sync.dma_start(out=wo_t[:, :], in_=WOP[nt, :, :].bitcast(F32R))
                for tc_ in range(8):
                    yps = ps.tile([128, 512], F32, tag="ps_proj", bufs=2)
                    for j in range(PAIRS):
                        nc.tensor.matmul(
                            yps[:, :],
                            aot[j][:, tc_ * 128:(tc_ + 1) * 128],
                            wo_t[:, j * 512:(j + 1) * 512],
                            start=(j == 0), stop=(j == PAIRS - 1),
                        )
                    y_sb = sb.tile([128, 512], F32, tag="y", bufs=2)
                    nc.vector.tensor_copy(y_sb[:, :], yps[:, :])
                    nc.sync.dma_start(
                        out=Y[tc_ * 128:(tc_ + 1) * 128, nt * 512:(nt + 1) * 512],
                        in_=y_sb[:, :],
                    )

    _split_multi_waits(nc)
    return nc


_nc_cache = {}
_last_results = None


def _get_nc(qh):
    if qh not in _nc_cache:
        _nc_cache[qh] = build_bass(qh)
    return _nc_cache[qh]


def _prep_weights(wq, bq, wk, bk, wv, bv, wo):
    wqT = np.ascontiguousarray(wq.T) * np.float32(1.0 / np.sqrt(DH))
    wkT = np.ascontiguousarray(wk.T)
    wvT = np.ascontiguousarray(wv.T)
    woT = np.ascontiguousarray(wo.T)
    # WQP[j, p, (d m)] = wqT[d*128+p, j*128+m]
    A = wqT.reshape(DINC, 128, PAIRS, 128)
    WQP = np.ascontiguousarray(A.transpose(2, 1, 0, 3).reshape(PAIRS, 128, 1024))
    A = wkT.reshape(DINC, 128, PAIRS, 128)
    WKP = np.ascontiguousarray(A.transpose(2, 1, 0, 3).reshape(PAIRS, 128, 1024))
    # WVP[blk, p, (d n)] = wvT[d*128+p, blk*256+n]
    A = wvT.reshape(DINC, 128, NBLK, 256)
    WVP = np.ascontiguousarray(A.transpose(2, 1, 0, 3).reshape(NBLK, 128, 2048))
    # WOP[nt, p, (j n)] = woT[j*128+p, nt*512+n]
    A = woT.reshape(PAIRS, 128, 2, 512)
    WOP = np.ascontiguousarray(A.transpose(2, 1, 0, 3).reshape(2, 128, 4096))
    bqs = (bq * np.float32(1.0 / np.sqrt(DH))).reshape(PAIRS, 128)
    bkr = bk.reshape(PAIRS, 128)
    BQK = np.empty((128, 16), np.float32)
    for jx in range(PAIRS):
        BQK[:, 2 * jx] = bqs[jx]
        BQK[:, 2 * jx + 1] = bkr[jx]
    BVB = np.ascontiguousarray(np.tile(bv.reshape(1, D), (128, 1)))
    return WQP, WKP, WVP, WOP, BQK, BVB


def kernel(x_input, wq, bq, wk, bk, wv, bv, wo, bo):
    x_input = np.asarray(x_input, dtype=np.float32)
    wq, bq = np.asarray(wq, np.float32), np.asarray(bq, np.float32)
    wk, bk = np.asarray(wk, np.float32), np.asarray(bk, np.float32)
    wv, bv = np.asarray(wv, np.float32), np.asarray(bv, np.float32)
    wo, bo = np.asarray(wo, np.float32), np.asarray(bo, np.float32)

    WQP, WKP, WVP, WOP, BQK, BVB = _prep_weights(wq, bq, wk, bk, wv, bv, wo)
    ONES2D = np.ones((128, 128), np.float32)

    shared = {
        "WQP": WQP, "WKP": WKP, "WVP": WVP, "WOP": WOP,
        "BQK": BQK, "BVB": BVB, "ONES2D": ONES2D,
    }
    xTs = [np.ascontiguousarray(x_input[b].T) for b in range(B)]

    # qh is baked into the program; all 8 cores must run ONE program under
    # SPMD, so instead bake qh=0 and shift each odd core's xT columns so its
    # queries sit at columns 0..1023 -- NO: that would break K/V (full seq).
    # Instead: build with qh as a parameter and run even/odd cores in one
    # launch is impossible under one NEFF; so we pass per-core xT where the
    # query half is ALWAYS columns [0,1024) by ROTATING the sequence for odd
    # cores, and un-rotate the keys... also breaks nothing: attention is
    # permutation-equivariant in keys! Rotating the key/token axis by 1024
    # for odd cores leaves softmax(QK^T)V unchanged per query; queries then
    # occupy columns 0..1023 of the rotated xT. Output rows are our queries
    # in rotated order = original columns 1024..2047. So: one program
    # (qh=0), odd cores get np.roll(xT, -1024, axis=1).
    nc = _get_nc(0)
    in_maps = []
    for c in range(N_CORES):
        b, qh = c // 2, c % 2
        xt = xTs[b] if qh == 0 else np.ascontiguousarray(
            np.roll(xTs[b], -NQ, axis=1))
        m = dict(shared)
        m["XT"] = xt
        in_maps.append(m)

    res = run_bass_kernel_spmd(nc, in_maps, list(range(N_CORES)))
    global _last_results
    _last_results = res

    out = np.empty((B, S, D), np.float32)
    for c in range(N_CORES):
        b, qh = c // 2, c % 2
        out[b, qh * NQ:(qh + 1) * NQ, :] = res.results[c]["Y"]
    out += bo.reshape(1, 1, D)
    return out
